# revision 1
# baseline (speedup 1.0000x reference)
"""Pointer-network LSTM decoder kernel for Trainium2 (Bass/Tile), SPMD over 8 cores.

Problem: B=32, S=1024, H=256 LSTM decoder with attention-pointer readout.
Per step: gates = x@W_ih.T + b_ih + h@W_hh.T + b_hh; LSTM cell; scores =
einsum('bsh,bh->bs', enc, h'); probs = softmax(scores); idx = argmax;
x_next = enc[idx]. Output: probs for all 1024 steps -> [B, S, S].

Key structure:
  - Data parallel over batch: 8 cores x 4 batch rows each, no collectives.
  - encW = enc @ W_ih.T + (b_ih + b_hh) precomputed once on device (in
    transposed [j, (b,s)] layout) so the per-step x-contribution becomes a
    row gather (by argmax index) instead of a matmul.
  - argmax(probs) == argmax(scores): softmax is off the critical path and
    batched 32 steps at a time in a [128, 1024] layout.
  - Per-step recurrent matmul h @ W_hh.T is done W-stationary producing
    gates directly in transposed [j, b] layout so the LSTM cell runs on
    [128, 8] tiles (full partition utilization).
  - Attention scores via column-tiled matmuls: col-group j computes batch
    j's scores with h broadcast to 32 columns (all PSUM partitions written,
    replicated), enabling max/max_index straight on the PSUM tile.
"""

import os
import sys
import numpy as np

sys.path.insert(0, "/opt/trn_rl_repo")

import concourse.bass as bass
import concourse.mybir as mybir
import concourse.tile as tile
from concourse import bacc
from concourse.bass_utils import run_bass_kernel_spmd

B, S, H = 32, 1024, 256
NCORES = 8
BL = B // NCORES  # batch rows per core
F32 = mybir.dt.float32
F32R = mybir.dt.float32r
AF = mybir.ActivationFunctionType
ALU = mybir.AluOpType
AX = mybir.AxisListType

# gate order in our layout: (i, f, o, g); reference W rows are (i, f, g, o)
GATE_PERM = np.concatenate(
    [np.arange(0, 512), np.arange(768, 1024), np.arange(512, 768)]
)

_CACHE = {}


def build_nc(T=S, f32r_attn=False, f32r_pre=False, ablate=()):
    """Build + schedule + compile the per-core Bass program (T decode steps).

    ablate: subset of {"argmax", "gather", "attn", "gates", "softmax"} —
    perf-bisection switches that skip pieces (breaking correctness).
    """
    ablate = set(ablate)
    nc = bacc.Bacc(
        "TRN2",
        target_bir_lowering=False,
        debug=False,
        num_devices=NCORES,
    )
    enc_d = nc.dram_tensor("enc", [BL, S, H], F32, kind="ExternalInput").ap()
    wiT_d = nc.dram_tensor("wiT", [128, 2048], F32, kind="ExternalInput").ap()
    whT_d = nc.dram_tensor("whT", [128, 2048], F32, kind="ExternalInput").ap()
    biasT_d = nc.dram_tensor("biasT", [128, 32], F32, kind="ExternalInput").ap()
    ident_d = nc.dram_tensor("ident", [128, 128], F32, kind="ExternalInput").ap()
    probs_d = nc.dram_tensor("probs", [BL, S, S], F32, kind="ExternalOutput").ap()
    # DRAM bounce buffer for per-step scores -> batched softmax blocks
    scratch_d = nc.dram_tensor("score_scratch", [BL, S, S], F32).ap()

    def mm_dt(ap, reduced):
        return ap.bitcast(F32R) if reduced else ap

    with tile.TileContext(nc) as tc:
        with tc.tile_pool(name="static", bufs=1) as st:
            encT = st.tile([128, BL * 2 * S], F32)   # [h_lo, (b, hh, s)]
            encWT = st.tile([128, 8 * BL * S], F32)  # [j_lo, (jc, b, s)]
            whT = st.tile([128, 2048], F32)          # [k_lo, (kc, jc*128)]
            biasT = st.tile([128, 32], F32)          # [j_lo, (jc, b)]
            c_sb = st.tile([128, 8], F32)            # [h_lo, (hh, b)]
            h_sb = st.tile([128, 8], F32)

            nc.sync.dma_start(whT[:, :], whT_d)
            nc.sync.dma_start(biasT[:, :], biasT_d)
            nc.gpsimd.memset(c_sb[:, :], 0.0)

            # ---- precompute encT (transpose enc into [h, s] layout) ----
            with (
                tc.tile_pool(name="pre_sb", bufs=3) as pre_sb,
                tc.tile_pool(name="pre_ps", bufs=2, space="PSUM") as pre_ps,
            ):
                ident = pre_sb.tile([128, 128], F32, tag="ident")
                nc.sync.dma_start(ident[:, :], ident_d)
                for b in range(BL):
                    for stile in range(S // 128):
                        raw = pre_sb.tile([128, H], F32, tag="raw")
                        nc.sync.dma_start(
                            raw[:, :], enc_d[b, stile * 128:(stile + 1) * 128, :]
                        )
                        for hh in range(2):
                            ps = pre_ps.tile([128, 128], F32, tag="tp", bufs=2)
                            nc.tensor.transpose(
                                ps[:, :], raw[:, hh * 128:(hh + 1) * 128], ident[:, :]
                            )
                            col = (b * 2 + hh) * S + stile * 128
                            nc.vector.tensor_copy(encT[:, col:col + 128], ps[:, :])

                # ---- precompute encWT = (enc @ W_ih.T + bias).T ----
                wiT = pre_sb.tile([128, 2048], F32, tag="wiT")
                nc.sync.dma_start(wiT[:, :], wiT_d)
                for jc in range(8):
                    for b in range(BL):
                        ps = pre_ps.tile([128, 1024], F32, tag="ew", bufs=2)
                        for kc in range(2):
                            for nh in range(2):
                                nc.tensor.matmul(
                                    ps[:, nh * 512:(nh + 1) * 512],
                                    mm_dt(wiT[:, kc * 1024 + jc * 128:
                                              kc * 1024 + (jc + 1) * 128], f32r_pre),
                                    mm_dt(encT[:, (b * 2 + kc) * S + nh * 512:
                                               (b * 2 + kc) * S + (nh + 1) * 512],
                                          f32r_pre),
                                    start=(kc == 0), stop=(kc == 1),
                                )
                        nc.scalar.activation(
                            encWT[:, (jc * BL + b) * S:(jc * BL + b + 1) * S],
                            ps[:, :],
                            AF.Identity,
                            bias=biasT[:, jc * 4:jc * 4 + 1],
                            scale=1.0,
                        )

            # ---- main decode loop ----
            with (
                tc.tile_pool(name="g_ps", bufs=2, space="PSUM") as g_pool,
                tc.tile_pool(name="s_ps", bufs=2, space="PSUM") as s_pool,
                tc.tile_pool(name="work", bufs=3) as work,
                tc.tile_pool(name="blocks", bufs=2) as blocks,
            ):
                rowbuf = None
                sBlock = None
                for t in range(T):
                    blk, tm = divmod(t, 32)
                    # -- gates --
                    gsb = work.tile([128, 32], F32, tag="gsb")
                    if t == 0 or "gates" in ablate:
                        nc.vector.tensor_copy(gsb[:, :], biasT[:, :])
                    else:
                        gps = g_pool.tile([128, 32], F32, tag="g")
                        for jc in range(8):
                            for kc in range(2):
                                nc.tensor.matmul(
                                    gps[:, jc * 4:(jc + 1) * 4],
                                    whT[:, kc * 1024 + jc * 128:
                                        kc * 1024 + (jc + 1) * 128],
                                    h_sb[:, kc * 4:(kc + 1) * 4],
                                    start=(kc == 0), stop=(kc == 1),
                                )
                        nc.vector.tensor_add(gsb[:, :], gps[:, :], rowbuf[:, :])
                    # -- LSTM cell (layout: i=0:8, f=8:16, o=16:24, g=24:32) --
                    nc.scalar.activation(gsb[:, 0:24], gsb[:, 0:24], AF.Sigmoid)
                    nc.scalar.activation(gsb[:, 24:32], gsb[:, 24:32], AF.Tanh)
                    ig = work.tile([128, 8], F32, tag="ig")
                    nc.vector.tensor_mul(ig[:, :], gsb[:, 0:8], gsb[:, 24:32])
                    nc.vector.tensor_mul(c_sb[:, :], gsb[:, 8:16], c_sb[:, :])
                    nc.vector.tensor_add(c_sb[:, :], c_sb[:, :], ig[:, :])
                    tcs = work.tile([128, 8], F32, tag="tcs")
                    nc.scalar.activation(tcs[:, :], c_sb[:, :], AF.Tanh)
                    nc.vector.tensor_mul(h_sb[:, :], gsb[:, 16:24], tcs[:, :])
                    # -- attention scores (col-tiled; batch j -> partitions 32j+) --
                    sps = s_pool.tile([128, 1024], F32, tag="s")
                    if "attn" not in ablate:
                        for j in range(BL):
                            for nh in range(2):
                                for kc in range(2):
                                    lhs = h_sb[:, kc * 4 + j:kc * 4 + j + 1]
                                    lhs = lhs.to_broadcast((128, 32))
                                    nc.tensor.matmul(
                                        sps[32 * j:32 * (j + 1),
                                            nh * 512:(nh + 1) * 512],
                                        mm_dt(lhs, f32r_attn),
                                        mm_dt(encT[:, (j * 2 + kc) * S + nh * 512:
                                                   (j * 2 + kc) * S + (nh + 1) * 512],
                                              f32r_attn),
                                        start=(kc == 0), stop=(kc == 1),
                                        tile_position=(0, 32 * j),
                                    )
                    # -- argmax (on PSUM; rows replicated within each 32-group) --
                    if "argmax" not in ablate:
                        maxv = work.tile([128, 8], F32, tag="maxv")
                        nc.vector.max(maxv[:, :], sps[:, :])
                        idx = work.tile([128, 8], mybir.dt.uint32, tag="idx")
                        nc.vector.max_index(idx[:, :], maxv[:, :], sps[:, :])
                    # -- stash scores for batched softmax --
                    # (DMA cannot read PSUM: bounce via an ACT copy to SBUF)
                    if tm == 0:
                        sBlock = blocks.tile([128, 1024], F32, tag="sb")
                        if T - blk * 32 < 32:  # partial tail block (small-T only)
                            nc.gpsimd.memset(sBlock[:, :], 0.0)
                    # Scores go SBUF -> DRAM scratch per step; each softmax
                    # block is read back with ONE DRAM->SBUF DMA. (A direct
                    # SBUF->SBUF partition shuffle left the DVE reader without
                    # DMA-queue waits in Tile's schedule -> race.)
                    if "softmax" not in ablate:
                        stg = work.tile([128, 1024], F32, tag="stg", bufs=2)
                        nc.scalar.copy(stg[:, :], sps[:, :])
                        nc.sync.dma_start(
                            scratch_d[:, t, :],
                            stg[:, :].rearrange("(a c) n -> a c n", c=32)[:, 0, :],
                        )
                    # -- gather encW rows for next step --
                    if "argmax" in ablate or "gather" in ablate:
                        rowbuf = biasT
                    elif t < T - 1:
                        rowbuf = work.tile([128, 32], F32, tag="row")
                        for b in range(BL):
                            rv = nc.values_load(
                                idx[32 * b:32 * b + 1, 0:1],
                                engines=[mybir.EngineType.Activation],
                                min_val=0, max_val=S - 1,
                                skip_runtime_bounds_check=True,
                            )
                            src = encWT[:, :].rearrange(
                                "p (j b s) -> p j b s", j=8, b=BL
                            )[:, :, b:b + 1, bass.ds(rv, 1)]
                            dst = rowbuf[:, :].rearrange(
                                "p (j b) -> p j b", j=8
                            )[:, :, b:b + 1]
                            nc.scalar.copy(dst, src)
                    # -- batched softmax + probs writeback --
                    if "softmax" in ablate:
                        continue
                    if tm == 31 or t == T - 1:
                        nsteps = tm + 1
                        sb_ap = sBlock[:, :]
                        # sBlock partition = b*32 + tm; per-batch DMAs keep
                        # every SBUF-side AP a plain partition range
                        for b in range(BL):
                            nc.sync.dma_start(
                                sBlock[b * 32:b * 32 + nsteps, :],
                                scratch_d[b, blk * 32:blk * 32 + nsteps, :],
                            )
                        bmax = work.tile([128, 1], F32, tag="bmax")
                        nc.vector.tensor_reduce(
                            bmax[:, :], sb_ap, axis=AX.X, op=ALU.max, negate=True
                        )
                        nc.scalar.activation(sb_ap, sb_ap, AF.Exp, bias=bmax[:, 0:1])
                        bsum = work.tile([128, 1], F32, tag="bsum")
                        nc.vector.tensor_reduce(
                            bsum[:, :], sb_ap, axis=AX.X, op=ALU.add
                        )
                        brec = work.tile([128, 1], F32, tag="brec")
                        nc.vector.reciprocal(brec[:, :], bsum[:, :])
                        nc.gpsimd.tensor_scalar_mul(sb_ap, sb_ap, brec[:, 0:1])
                        for b in range(BL):
                            nc.sync.dma_start(
                                probs_d[b, blk * 32:blk * 32 + nsteps, :],
                                sBlock[b * 32:b * 32 + nsteps, :],
                            )

    nc.compile()
    return nc


def _host_inputs(encoder_outputs, W_ih, W_hh, b_ih, b_hh):
    """Pure layout prep (weight transposes/permutes) on host."""
    enc = np.ascontiguousarray(np.asarray(encoder_outputs, dtype=np.float32))
    W_ih = np.asarray(W_ih, dtype=np.float32)[GATE_PERM]
    W_hh = np.asarray(W_hh, dtype=np.float32)[GATE_PERM]
    bias = (np.asarray(b_ih, dtype=np.float32)
            + np.asarray(b_hh, dtype=np.float32))[GATE_PERM]

    def t_tiles(W):  # [1024, 256] -> [128, (kc 2, jc 8)*128] with W.T tiling
        out = np.empty((128, 2048), np.float32)
        WT = W.T  # [256, 1024]
        for kc in range(2):
            for jc in range(8):
                out[:, kc * 1024 + jc * 128:kc * 1024 + (jc + 1) * 128] = \
                    WT[kc * 128:(kc + 1) * 128, jc * 128:(jc + 1) * 128]
        return np.ascontiguousarray(out)

    wiT = t_tiles(W_ih)
    whT = t_tiles(W_hh)
    biasT = np.empty((128, 32), np.float32)
    for jc in range(8):
        for b in range(BL):
            biasT[:, jc * 4 + b] = bias[jc * 128:(jc + 1) * 128]
    ident = np.eye(128, dtype=np.float32)

    in_maps = []
    for c in range(NCORES):
        in_maps.append({
            "enc": enc[c * BL:(c + 1) * BL],
            "wiT": wiT,
            "whT": whT,
            "biasT": biasT,
            "ident": ident,
        })
    return in_maps


def kernel(encoder_outputs, W_ih, W_hh, b_ih, b_hh):
    key = "nc"
    if key not in _CACHE:
        _CACHE[key] = build_nc(
            T=S,
            f32r_attn=os.environ.get("PTR_F32R", "0") == "1",
            f32r_pre=os.environ.get("PTR_F32R", "0") == "1",
        )
    nc = _CACHE[key]
    in_maps = _host_inputs(encoder_outputs, W_ih, W_hh, b_ih, b_hh)
    res = run_bass_kernel_spmd(nc, in_maps, list(range(NCORES)))
    out = np.concatenate([res.results[c]["probs"] for c in range(NCORES)], axis=0)
    return out.astype(np.float32)



# revision 13
# speedup vs baseline: 1.0736x; 1.0736x over previous
"""Pointer-network LSTM decoder kernel for Trainium2 (Bass/Tile), SPMD over 8 cores.

Problem: B=32, S=1024, H=256 LSTM decoder with attention-pointer readout.
Per step: gates = x@W_ih.T + b_ih + h@W_hh.T + b_hh; LSTM cell; scores =
einsum('bsh,bh->bs', enc, h'); probs = softmax(scores); idx = argmax;
x_next = enc[idx]. Output: probs for all 1024 steps -> [B, S, S].

v2 architecture (vs v1 baseline):
  - Data parallel over batch: 8 cores x 4 batch rows, no collectives.
  - encW = enc @ W_ih.T + bias precomputed once (transposed layout) so the
    per-step x contribution is a row gather by argmax index.
  - The 4 batch rows are split into TWO pipelined groups (A: b0-1, B: b2-3).
    Group X's serial tail (argmax -> gather -> cell) hides under the other
    group's attention matmul streaming, keeping the PE array busy (and its
    HAM clock-gate warm) continuously.
  - Per-step softmax is GONE from the loop: h is streamed to a DRAM history
    buffer, and probs = softmax(enc @ h_hist.T) are recomputed in a batched
    end-pass (one [t=128, s=1024] score block per (batch, ttile)) that is
    emitted interleaved with the loop so it fills engine idle slots.
  - Everything stays fp32: empirically the min top-2 score gap along the
    trajectory is 6.8e-5 sigma, so reduced-precision scores (bf16/f32r)
    would flip argmaxes and diverge the whole trajectory.
"""

import os
import sys
import numpy as np

sys.path.insert(0, "/opt/trn_rl_repo")

import concourse.bass as bass
import concourse.mybir as mybir
import concourse.tile as tile
from concourse import bacc
from concourse.bass_utils import run_bass_kernel_spmd

B, S, H = 32, 1024, 256
NCORES = 8
BL = B // NCORES  # batch rows per core
F32 = mybir.dt.float32
AF = mybir.ActivationFunctionType
ALU = mybir.AluOpType
AX = mybir.AxisListType

# gate order in our layout: (i, f, o, g); reference W rows are (i, f, g, o)
GATE_PERM = np.concatenate(
    [np.arange(0, 512), np.arange(768, 1024), np.arange(512, 768)]
)

_CACHE = {}


def build_nc(T=S):
    """Build + schedule + compile the per-core Bass program (T decode steps)."""
    nc = bacc.Bacc(
        "TRN2",
        target_bir_lowering=False,
        debug=False,
        num_devices=NCORES,
    )
    enc_d = nc.dram_tensor("enc", [BL, S, H], F32, kind="ExternalInput").ap()
    wiT_d = nc.dram_tensor("wiT", [128, 2048], F32, kind="ExternalInput").ap()
    whT_d = nc.dram_tensor("whT", [128, 2048], F32, kind="ExternalInput").ap()
    biasT_d = nc.dram_tensor("biasT", [128, 32], F32, kind="ExternalInput").ap()
    ident_d = nc.dram_tensor("ident", [128, 128], F32, kind="ExternalInput").ap()
    probs_d = nc.dram_tensor("probs", [BL, S, S], F32, kind="ExternalOutput").ap()
    # h history: hist[t, p, hh*4 + b] = h_t[hh*128 + p, b]
    hist_d = nc.dram_tensor("hist", [T, 128, 8], F32).ap()

    GROUPS = ((0, 1), (2, 3))  # global batch rows per group

    with tile.TileContext(nc) as tc:
        with tc.tile_pool(name="static", bufs=1) as st:
            encT = st.tile([128, BL * 2 * S], F32)   # [h_lo, (b, hh, s)]
            encWT = st.tile([128, 8 * BL * S], F32)  # [j_lo, (jc, b, s)]
            whT = st.tile([128, 2048], F32)          # [k_lo, (kc, jc*128)]
            biasT = st.tile([128, 32], F32)          # [j_lo, (jc, b)]
            cA = st.tile([128, 4], F32)              # [h_lo, (hh, bl)]
            cB = st.tile([128, 4], F32)
            hA = st.tile([128, 4], F32)
            hB = st.tile([128, 4], F32)
            h_of = {0: hA, 1: hB}
            c_of = {0: cA, 1: cB}

            nc.sync.dma_start(whT[:, :], whT_d)
            nc.sync.dma_start(biasT[:, :], biasT_d)
            nc.gpsimd.memset(cA[:, :], 0.0)
            nc.gpsimd.memset(cB[:, :], 0.0)

            # ---- precompute encT (transpose enc into [h, s] layout) ----
            with (
                tc.tile_pool(name="pre_sb", bufs=3) as pre_sb,
                tc.tile_pool(name="pre_ps", bufs=2, space="PSUM") as pre_ps,
            ):
                ident = pre_sb.tile([128, 128], F32, tag="ident")
                nc.sync.dma_start(ident[:, :], ident_d)
                for b in range(BL):
                    for stile in range(S // 128):
                        raw = pre_sb.tile([128, H], F32, tag="raw")
                        nc.sync.dma_start(
                            raw[:, :], enc_d[b, stile * 128:(stile + 1) * 128, :]
                        )
                        for hh in range(2):
                            ps = pre_ps.tile([128, 128], F32, tag="tp", bufs=2)
                            nc.tensor.transpose(
                                ps[:, :], raw[:, hh * 128:(hh + 1) * 128], ident[:, :]
                            )
                            col = (b * 2 + hh) * S + stile * 128
                            nc.vector.tensor_copy(encT[:, col:col + 128], ps[:, :])

                # ---- precompute encWT = (enc @ W_ih.T + bias).T ----
                wiT = pre_sb.tile([128, 2048], F32, tag="wiT")
                nc.sync.dma_start(wiT[:, :], wiT_d)
                for jc in range(8):
                    for b in range(BL):
                        ps = pre_ps.tile([128, 1024], F32, tag="ew", bufs=2)
                        for kc in range(2):
                            for nh in range(2):
                                nc.tensor.matmul(
                                    ps[:, nh * 512:(nh + 1) * 512],
                                    wiT[:, kc * 1024 + jc * 128:
                                        kc * 1024 + (jc + 1) * 128],
                                    encT[:, (b * 2 + kc) * S + nh * 512:
                                         (b * 2 + kc) * S + (nh + 1) * 512],
                                    start=(kc == 0), stop=(kc == 1),
                                )
                        nc.scalar.activation(
                            encWT[:, (jc * BL + b) * S:(jc * BL + b + 1) * S],
                            ps[:, :],
                            AF.Identity,
                            bias=biasT[:, jc * 4:jc * 4 + 1],
                            scale=1.0,
                        )

            # ---- main decode loop ----
            with (
                tc.tile_pool(name="g_ps", bufs=2, space="PSUM") as g_pool,
                tc.tile_pool(name="s_ps", bufs=2, space="PSUM") as s_pool,
                tc.tile_pool(name="pp_ps", bufs=2, space="PSUM") as pp_pool,
                tc.tile_pool(name="work", bufs=3) as work,
                tc.tile_pool(name="hb", bufs=2) as hb_pool,
                tc.tile_pool(name="sb", bufs=2) as sb_pool,
            ):
                biasT3 = biasT[:, :].rearrange("p (j b) -> p j b", j=8)
                encWT4 = encWT[:, :].rearrange("p (j b s) -> p j b s", j=8, b=BL)

                def hist_dst(t, g):
                    return hist_d[t].rearrange("p (hh b) -> p hh b", hh=2)[
                        :, :, 2 * g:2 * g + 2]

                def cell(g, gsb, t):
                    """LSTM cell for group g from pre-activation gsb [128,16];
                    writes c/h and streams h to hist."""
                    hX, cX = h_of[g], c_of[g]
                    nc.scalar.activation(gsb[:, 0:12], gsb[:, 0:12], AF.Sigmoid)
                    nc.scalar.activation(gsb[:, 12:16], gsb[:, 12:16], AF.Tanh)
                    ig = work.tile([128, 4], F32, tag=f"ig{g}")
                    nc.vector.tensor_mul(ig[:, :], gsb[:, 0:4], gsb[:, 12:16])
                    nc.vector.tensor_mul(cX[:, :], gsb[:, 4:8], cX[:, :])
                    nc.vector.tensor_add(cX[:, :], cX[:, :], ig[:, :])
                    tcs = work.tile([128, 4], F32, tag=f"tcs{g}")
                    nc.scalar.activation(tcs[:, :], cX[:, :], AF.Tanh)
                    nc.vector.tensor_mul(hX[:, :], gsb[:, 8:12], tcs[:, :])
                    nc.sync.dma_start(
                        hist_dst(t, g),
                        hX[:, :].rearrange("p (hh b) -> p hh b", hh=2),
                    )

                def endpass_block(k, bg, nsteps):
                    """probs[bg, 128k:128k+nsteps, :] from the h history."""
                    hblk = hb_pool.tile([128, 1024], F32, tag="hblk")
                    if nsteps < 128:
                        nc.gpsimd.memset(hblk[:, :], 0.0)
                    nc.sync.dma_start(
                        hblk[:, 0:nsteps * 8].rearrange("p (t c) -> p t c", c=8),
                        hist_d[k * 128:k * 128 + nsteps].rearrange(
                            "t p c -> p t c"),
                    )
                    hblk3 = hblk[:, :].rearrange("p (t c) -> p t c", c=8)
                    pps = []
                    for nh in range(2):
                        pp = pp_pool.tile([128, 512], F32, tag="pp")
                        for hh in range(2):
                            nc.tensor.matmul(
                                pp[:, :],
                                hblk3[:, :, hh * 4 + bg],
                                encT[:, (bg * 2 + hh) * S + nh * 512:
                                     (bg * 2 + hh) * S + (nh + 1) * 512],
                                start=(hh == 0), stop=(hh == 1),
                            )
                        pps.append(pp)
                    nmx0 = work.tile([128, 1], F32, tag="nmx0")
                    nmx1 = work.tile([128, 1], F32, tag="nmx1")
                    nc.vector.tensor_reduce(
                        nmx0[:, :], pps[0][:, :], axis=AX.X, op=ALU.max, negate=True)
                    nc.vector.tensor_reduce(
                        nmx1[:, :], pps[1][:, :], axis=AX.X, op=ALU.max, negate=True)
                    nc.vector.tensor_tensor(
                        nmx0[:, :], nmx0[:, :], nmx1[:, :], ALU.min)
                    sblk = sb_pool.tile([128, 1024], F32, tag="sblk")
                    for nh in range(2):
                        nc.scalar.activation(
                            sblk[:, nh * 512:(nh + 1) * 512], pps[nh][:, :],
                            AF.Exp, bias=nmx0[:, 0:1])
                    bsum = work.tile([128, 1], F32, tag="bsum")
                    nc.vector.tensor_reduce(
                        bsum[:, :], sblk[:, :], axis=AX.X, op=ALU.add)
                    brec = work.tile([128, 1], F32, tag="brec")
                    nc.vector.reciprocal(brec[:, :], bsum[:, :])
                    nc.vector.tensor_scalar_mul(sblk[:, :], sblk[:, :], brec[:, 0:1])
                    nc.sync.dma_start(
                        probs_d[bg, k * 128:k * 128 + nsteps, :],
                        sblk[0:nsteps, :])

                # prologue: step-0 cell from bias only (x=0, h=0)
                for g, rows in enumerate(GROUPS):
                    gsb = work.tile([128, 16], F32, tag=f"gsb{g}")
                    nc.vector.tensor_copy(
                        gsb[:, :].rearrange("p (j b) -> p j b", j=8),
                        biasT3[:, :, rows[0]:rows[0] + 2],
                    )
                    cell(g, gsb, 0)

                rowbuf = {0: None, 1: None}
                for t in range(T - 1):
                    for g, rows in enumerate(GROUPS):
                        hX = h_of[g]
                        # -- attention scores for step t --
                        sps = s_pool.tile([128, 1024], F32, tag="s")
                        for bl in range(2):
                            bg = rows[bl]
                            for nh in range(2):
                                for kc in range(2):
                                    lhs = hX[:, kc * 2 + bl:kc * 2 + bl + 1]
                                    lhs = lhs.to_broadcast((128, 32))
                                    nc.tensor.matmul(
                                        sps[32 * bl:32 * (bl + 1),
                                            nh * 512:(nh + 1) * 512],
                                        lhs,
                                        encT[:, (bg * 2 + kc) * S + nh * 512:
                                             (bg * 2 + kc) * S + (nh + 1) * 512],
                                        start=(kc == 0), stop=(kc == 1),
                                        tile_position=(0, 32 * bl),
                                    )
                        # -- argmax (only partitions 0:64 were written) --
                        maxv = work.tile([128, 8], F32, tag=f"maxv{g}")
                        nc.vector.max(maxv[0:64, :], sps[0:64, :])
                        idx = work.tile([128, 8], mybir.dt.uint32, tag=f"idx{g}")
                        nc.vector.max_index(
                            idx[0:64, :], maxv[0:64, :], sps[0:64, :])
                        # -- gather encW rows for step t+1 --
                        rb = work.tile([128, 16], F32, tag=f"row{g}")
                        rb3 = rb[:, :].rearrange("p (j b) -> p j b", j=8)
                        for bl in range(2):
                            bg = rows[bl]
                            rv = nc.values_load(
                                idx[32 * bl:32 * bl + 1, 0:1],
                                engines=[mybir.EngineType.Activation],
                                min_val=0, max_val=S - 1,
                                skip_runtime_bounds_check=True,
                            )
                            nc.scalar.copy(
                                rb3[:, :, bl:bl + 1],
                                encWT4[:, :, bg:bg + 1, bass.ds(rv, 1)],
                            )
                        rowbuf[g] = rb
                        # -- gates matmul for step t+1 --
                        gps = g_pool.tile([128, 16], F32, tag="g")
                        for jc in range(8):
                            for kc in range(2):
                                nc.tensor.matmul(
                                    gps[:, jc * 2:(jc + 1) * 2],
                                    whT[:, kc * 1024 + jc * 128:
                                        kc * 1024 + (jc + 1) * 128],
                                    hX[:, kc * 2:(kc + 1) * 2],
                                    start=(kc == 0), stop=(kc == 1),
                                )
                        # -- cell for step t+1 --
                        gsb = work.tile([128, 16], F32, tag=f"gsb{g}")
                        nc.vector.tensor_add(gsb[:, :], gps[:, :], rowbuf[g][:, :])
                        cell(g, gsb, t + 1)
                    # -- spread the probs end-pass across the loop --
                    if t >= 128 and t % 32 == 0:
                        endpass_block(t // 128 - 1, (t % 128) // 32, 128)

                # remaining end-pass blocks
                done = set()
                for t in range(T - 1):
                    if t >= 128 and t % 32 == 0:
                        done.add((t // 128 - 1, (t % 128) // 32))
                nt = T // 128 + (1 if T % 128 else 0)
                for k in range(nt):
                    for bg in range(BL):
                        if (k, bg) not in done:
                            endpass_block(k, bg, min(128, T - k * 128))

    nc.compile()
    return nc


def _host_inputs(encoder_outputs, W_ih, W_hh, b_ih, b_hh):
    """Pure layout prep (weight transposes/permutes) on host."""
    enc = np.ascontiguousarray(np.asarray(encoder_outputs, dtype=np.float32))
    W_ih = np.asarray(W_ih, dtype=np.float32)[GATE_PERM]
    W_hh = np.asarray(W_hh, dtype=np.float32)[GATE_PERM]
    bias = (np.asarray(b_ih, dtype=np.float32)
            + np.asarray(b_hh, dtype=np.float32))[GATE_PERM]

    def t_tiles(W):  # [1024, 256] -> [128, (kc 2, jc 8)*128] with W.T tiling
        out = np.empty((128, 2048), np.float32)
        WT = W.T  # [256, 1024]
        for kc in range(2):
            for jc in range(8):
                out[:, kc * 1024 + jc * 128:kc * 1024 + (jc + 1) * 128] = \
                    WT[kc * 128:(kc + 1) * 128, jc * 128:(jc + 1) * 128]
        return np.ascontiguousarray(out)

    wiT = t_tiles(W_ih)
    whT = t_tiles(W_hh)
    biasT = np.empty((128, 32), np.float32)
    for jc in range(8):
        for b in range(BL):
            biasT[:, jc * 4 + b] = bias[jc * 128:(jc + 1) * 128]
    ident = np.eye(128, dtype=np.float32)

    in_maps = []
    for c in range(NCORES):
        in_maps.append({
            "enc": enc[c * BL:(c + 1) * BL],
            "wiT": wiT,
            "whT": whT,
            "biasT": biasT,
            "ident": ident,
        })
    return in_maps


def kernel(encoder_outputs, W_ih, W_hh, b_ih, b_hh):
    key = "nc"
    if key not in _CACHE:
        _CACHE[key] = build_nc(T=S)
    nc = _CACHE[key]
    in_maps = _host_inputs(encoder_outputs, W_ih, W_hh, b_ih, b_hh)
    res = run_bass_kernel_spmd(nc, in_maps, list(range(NCORES)))
    out = np.concatenate([res.results[c]["probs"] for c in range(NCORES)], axis=0)
    return out.astype(np.float32)
